# revision 4
# baseline (speedup 1.0000x reference)
"""EqualizedModulatedConv2d (StyleGAN2-style modulated conv) on 8 Trainium2 cores.

Reference computation (per sample n):
    mod[n, ic]  = (style[n] @ fc_weight.T) * FC_SCALER + fc_bias + 1
    w[n]        = WEIGHT_SCALER * weight * mod[n, :, None, None]          # [oC, iC, 3, 3]
    demod[n,oc] = rsqrt(sum_{ic,kh,kw} w^2 + 1e-8)
    out[n]      = conv2d(x[n], w[n] * demod[n, :, None, None, None], pad=1)

Key identity used on device: the conv is linear, so
    out[n, oc] = (WEIGHT_SCALER * demod[n, oc]) * conv2d(x[n] * mod[n, ic], weight)
with
    WEIGHT_SCALER * demod[n, oc] = 1 / sqrt(sumsq[n, oc] + 1e-8 / WEIGHT_SCALER^2)
    sumsq[n, oc] = sum_ic A[ic, oc] * mod[n, ic]^2,   A[ic, oc] = sum_taps weight^2

Sharding: data-parallel over N (16 samples / 8 cores = 2 samples per core);
weight / fc_weight replicated. The conv itself runs as 3x3 = 9 shifted matmuls
on the PE array, contracting over iC in 4 chunks of 128 partitions, from a
zero-padded 34x34 SBUF image buffer.
"""

import os

import numpy as np

import concourse.bass as bass
import concourse.tile as tile
from concourse import bacc, mybir
import concourse.bass_utils as bass_utils

# ---- problem constants (hardcoded per the harness contract) ----
N, IC, OC, K, SDIM, H, W = 16, 512, 512, 3, 512, 32, 32
N_CORES = 8
NPC = N // N_CORES            # samples per core = 2
PC = IC // 128                # ic chunks = 4
OCC = OC // 128               # oc chunks = 4
SC = SDIM // 128              # sdim chunks = 4
NTAP = K * K                  # 9
HP, WP = H + 2, W + 2         # 34, 34 padded
FC_SCALER = 1.0 / np.sqrt(SDIM)
WEIGHT_SCALER = 1.0 / np.sqrt(IC * K * K)
DEMOD_EPS = 1e-8 / (WEIGHT_SCALER * WEIGHT_SCALER)   # 1e-8 * IC * K * K
HALF = 16                     # rows per conv matmul (16*32 = 512 = PSUM bank)

# matmul operand mode: "f32" (exact, 4 cyc/row), "f32r" (fast fp32, 1 cyc/row),
# "bf16" (fast + halved weight-load cost, lower precision)
MODE = os.environ.get("TRN_CONV_DTYPE", "f32r")

_NC_CACHE = {}
LAST_RESULT = None  # test.py reads exec_time_ns off this


def _mm(ap):
    """Cast an SBUF view to the matmul streaming dtype for the current mode."""
    if MODE == "f32r":
        return ap.bitcast(mybir.dt.float32r)
    return ap


def build_nc(mode=None):
    mode = mode or MODE
    if mode in _NC_CACHE:
        return _NC_CACHE[mode]

    f32 = mybir.dt.float32
    conv_dt = mybir.dt.bfloat16 if mode == "bf16" else f32

    nc = bacc.Bacc("TRN2", target_bir_lowering=False, debug=False,
                   num_devices=N_CORES)

    x = nc.dram_tensor("x", [NPC, IC, H, W], f32, kind="ExternalInput").ap()
    stt = nc.dram_tensor("stt", [SDIM, NPC], f32, kind="ExternalInput").ap()
    fcwt = nc.dram_tensor("fcwt", [SDIM, IC], f32, kind="ExternalInput").ap()
    fcb = nc.dram_tensor("fcb", [IC], f32, kind="ExternalInput").ap()
    wt = nc.dram_tensor("wt", [IC, NTAP, OC], f32, kind="ExternalInput").ap()
    y = nc.dram_tensor("y", [NPC, OC, H, W], f32, kind="ExternalOutput").ap()

    xr = x.rearrange("n (c p) h w -> n c p h w", p=128)
    str_ = stt.rearrange("(sc p) n -> p sc n", p=128)
    fcr = fcwt.rearrange("(sc p) i -> sc p i", p=128)
    fbr = fcb.rearrange("(c p) -> p c", p=128)
    wr = wt.rearrange("(c p) t o -> c p t o", p=128)
    yr = y.rearrange("n (o p) h w -> n o p (h w)", p=128)

    with tile.TileContext(nc) as tc:
        import contextlib
        with contextlib.ExitStack() as ctx:
            singles = ctx.enter_context(tc.tile_pool(name="singles", bufs=1))
            small = ctx.enter_context(tc.tile_pool(name="small", bufs=4))
            sq = ctx.enter_context(tc.tile_pool(name="sq", bufs=1))
            outp = ctx.enter_context(tc.tile_pool(name="outp", bufs=6))
            psc = ctx.enter_context(tc.tile_pool(name="psc", bufs=5, space="PSUM"))
            pss = ctx.enter_context(tc.tile_pool(name="pss", bufs=2, space="PSUM"))
            stage = None
            if mode == "bf16":
                stage = ctx.enter_context(tc.tile_pool(name="stage", bufs=2))
                xnp = ctx.enter_context(tc.tile_pool(name="xnp", bufs=2))

            # ---- persistent SBUF tensors ----
            fcw_sb = singles.tile([128, SC, IC], f32)
            st_sb = singles.tile([128, SC, NPC], f32)
            fb_sb = singles.tile([128, PC], f32)
            modT_sb = singles.tile([128, PC, NPC], f32)
            mod2T_sb = singles.tile([128, PC, NPC], f32)
            A_sb = singles.tile([128, PC, OC], f32)
            demodT_sb = singles.tile([128, OCC, NPC], f32)
            wt_sb = singles.tile([128, PC, NTAP, OC], conv_dt)
            xs_pad = singles.tile([128, NPC, PC, HP, WP], conv_dt)

            eps_sb = singles.tile([128, 1], f32)
            nc.vector.memset(eps_sb[:], float(DEMOD_EPS))

            # ---- style modulation: modT[ic, n] ----
            nc.sync.dma_start(st_sb[:], str_)
            nc.sync.dma_start(fb_sb[:], fbr)
            for sc in range(SC):
                nc.sync.dma_start(fcw_sb[:, sc], fcr[sc])
            nc.vector.tensor_scalar_add(fb_sb[:], fb_sb[:], 1.0)
            for c in range(PC):
                pm = pss.tile([128, NPC], f32, tag="pmm")
                for sc in range(SC):
                    nc.tensor.matmul(
                        pm[:], fcw_sb[:, sc, c * 128:(c + 1) * 128],
                        st_sb[:, sc], start=(sc == 0), stop=(sc == SC - 1))
                nc.scalar.activation(
                    modT_sb[:, c], pm[:], mybir.ActivationFunctionType.Identity,
                    bias=fb_sb[:, c:c + 1], scale=FC_SCALER)
            nc.vector.tensor_mul(mod2T_sb[:], modT_sb[:], modT_sb[:])

            # ---- padded, modulated input images ----
            nc.gpsimd.memset(xs_pad[:], 0.0)
            for n in range(NPC):
                if mode == "bf16":
                    xn = xnp.tile([128, PC, H, W], f32, tag="xn")
                    nc.sync.dma_start(
                        xn[:], x.rearrange("n (c p) h w -> n p c h w", p=128)[n])
                    for c in range(PC):
                        nc.scalar.mul(xs_pad[:, n, c, 1:H + 1, 1:W + 1],
                                      xn[:, c], modT_sb[:, c, n:n + 1])
                else:
                    for c in range(PC):
                        iv = xs_pad[:, n, c, 1:H + 1, 1:W + 1]
                        nc.sync.dma_start(iv, xr[n, c])
                        nc.scalar.mul(iv, iv, modT_sb[:, c, n:n + 1])

            # ---- weights + demodulation ----
            for c in range(PC):
                if mode == "bf16":
                    ws = stage.tile([128, NTAP, OC], f32, tag="ws")
                    nc.sync.dma_start(ws[:], wr[c])
                    nc.scalar.copy(wt_sb[:, c], ws[:])  # cast to bf16
                    src = ws
                else:
                    nc.sync.dma_start(wt_sb[:, c], wr[c])
                    src = wt_sb[:, c]
                w2 = sq.tile([128, NTAP, OC], f32, tag="w2")
                nc.vector.tensor_mul(w2[:], src[:], src[:])
                nc.vector.reduce_sum(A_sb[:, c], w2.rearrange("p t o -> p o t"),
                                     axis=mybir.AxisListType.X)
            for o in range(OCC):
                ps2 = pss.tile([128, NPC], f32, tag="pmm")
                for c in range(PC):
                    nc.tensor.matmul(
                        ps2[:], A_sb[:, c, o * 128:(o + 1) * 128],
                        mod2T_sb[:, c], start=(c == 0), stop=(c == PC - 1))
                sqt = small.tile([128, NPC], f32, tag="sqt")
                nc.scalar.activation(sqt[:], ps2[:],
                                     mybir.ActivationFunctionType.Sqrt,
                                     bias=eps_sb[:])
                nc.vector.reciprocal(demodT_sb[:, o], sqt[:])

            # ---- the conv: 2 samples x 4 oc-chunks x 2 half-images,
            #      each accumulating 4 ic-chunks x 9 taps ----
            for n in range(NPC):
                for o in range(OCC):
                    for h in range(H // HALF):
                        ps = psc.tile([128, HALF * W], f32, tag="ps")
                        i = 0
                        for c in range(PC):
                            for ky in range(K):
                                for kx in range(K):
                                    nc.tensor.matmul(
                                        ps[:],
                                        _mm(wt_sb[:, c, ky * K + kx,
                                                  o * 128:(o + 1) * 128]),
                                        _mm(xs_pad[:, n, c,
                                                   ky + h * HALF:ky + h * HALF + HALF,
                                                   kx:kx + W]),
                                        start=(i == 0), stop=(i == PC * NTAP - 1))
                                    i += 1
                        ob = outp.tile([128, HALF * W], f32, tag="ob")
                        nc.scalar.copy(ob[:], ps[:])
                        nc.vector.tensor_scalar_mul(ob[:], ob[:],
                                                    demodT_sb[:, o, n:n + 1])
                        nc.sync.dma_start(
                            yr[n, o][:, h * HALF * W:(h + 1) * HALF * W], ob[:])

    nc.finalize()
    _NC_CACHE[mode] = nc
    return nc


def _shard_inputs(x, style, weight, fc_weight, fc_bias):
    f = np.float32
    wt_host = np.ascontiguousarray(
        weight.astype(f).transpose(1, 2, 3, 0).reshape(IC, NTAP, OC))
    fcwt_host = np.ascontiguousarray(fc_weight.astype(f).T)
    fcb_host = np.ascontiguousarray(fc_bias.astype(f))
    in_maps = []
    for i in range(N_CORES):
        sl = slice(i * NPC, (i + 1) * NPC)
        in_maps.append({
            "x": np.ascontiguousarray(x[sl].astype(f)),
            "stt": np.ascontiguousarray(style[sl].astype(f).T),
            "fcwt": fcwt_host,
            "fcb": fcb_host,
            "wt": wt_host,
        })
    return in_maps


def kernel(x, style, weight, fc_weight, fc_bias):
    global LAST_RESULT
    x = np.asarray(x)
    style = np.asarray(style)
    weight = np.asarray(weight)
    fc_weight = np.asarray(fc_weight)
    fc_bias = np.asarray(fc_bias)

    nc = build_nc()
    in_maps = _shard_inputs(x, style, weight, fc_weight, fc_bias)
    res = bass_utils.run_bass_kernel_spmd(
        nc, in_maps, core_ids=list(range(N_CORES)))
    LAST_RESULT = res
    out = np.concatenate([res.results[i]["y"] for i in range(N_CORES)], axis=0)
    return out.astype(np.float32)


# revision 23
# speedup vs baseline: 3.1415x; 3.1415x over previous
"""EqualizedModulatedConv2d (StyleGAN2-style modulated conv) on 8 Trainium2 cores.

Reference computation (per sample n):
    mod[n, ic]  = (style[n] @ fc_weight.T) * FC_SCALER + fc_bias + 1
    w[n]        = WEIGHT_SCALER * weight * mod[n, :, None, None]          # [oC, iC, 3, 3]
    demod[n,oc] = rsqrt(sum_{ic,kh,kw} w^2 + 1e-8)
    out[n]      = conv2d(x[n], w[n] * demod[n, :, None, None, None], pad=1)

Key identity used on device: the conv is linear, so
    out[n, oc] = (WEIGHT_SCALER * demod[n, oc]) * conv2d(x[n] * mod[n, ic], weight)
with
    WEIGHT_SCALER * demod[n, oc] = 1 / sqrt(sumsq[n, oc] + 1e-8 / WEIGHT_SCALER^2)
    sumsq[n, oc] = sum_ic A[ic, oc] * mod[n, ic]^2,   A[ic, oc] = sum_taps weight^2

Sharding: data-parallel over N (16 samples / 8 cores = 2 samples per core);
weight / fc_weight replicated. The conv itself runs as 3x3 = 9 shifted matmuls
on the PE array, contracting over iC in 4 chunks of 128 partitions, from a
zero-padded 34x34 SBUF image buffer.
"""

import os

import numpy as np

import concourse.bass as bass
import concourse.tile as tile
from concourse import bacc, mybir
import concourse.bass_utils as bass_utils

# ---- problem constants (hardcoded per the harness contract) ----
N, IC, OC, K, SDIM, H, W = 16, 512, 512, 3, 512, 32, 32
N_CORES = 8
NPC = N // N_CORES            # samples per core = 2
PC = IC // 128                # ic chunks = 4
OCC = OC // 128               # oc chunks = 4
SC = SDIM // 128              # sdim chunks = 4
NTAP = K * K                  # 9
HP, WP = H + 2, W + 2         # 34, 34 padded
FC_SCALER = 1.0 / np.sqrt(SDIM)
WEIGHT_SCALER = 1.0 / np.sqrt(IC * K * K)
DEMOD_EPS = 1e-8 / (WEIGHT_SCALER * WEIGHT_SCALER)   # 1e-8 * IC * K * K
HALF = 16                     # rows per conv matmul (16*32 = 512 = PSUM bank)

# matmul operand mode: "f32" (exact, 4 cyc/row), "f32r" (fast fp32, 1 cyc/row),
# "bf16" (fast + halved weight-load cost, lower precision)
MODE = os.environ.get("TRN_CONV_DTYPE", "f32r")

_NC_CACHE = {}
LAST_RESULT = None  # test.py reads exec_time_ns off this


def build_nc(mode=None):
    mode = mode or MODE
    if mode in _NC_CACHE:
        return _NC_CACHE[mode]

    f32 = mybir.dt.float32
    conv_dt = {"bf16": mybir.dt.bfloat16,
               "f32r": mybir.dt.float32r,
               "f32": f32}[mode]
    staged = mode != "f32"

    nc = bacc.Bacc("TRN2", target_bir_lowering=False, debug=False,
                   num_devices=N_CORES)

    x = nc.dram_tensor("x", [NPC, IC, H, W], f32, kind="ExternalInput").ap()
    stt = nc.dram_tensor("stt", [SDIM, NPC], f32, kind="ExternalInput").ap()
    fcwt = nc.dram_tensor("fcwt", [SDIM, IC], f32, kind="ExternalInput").ap()
    fcb = nc.dram_tensor("fcb", [IC], f32, kind="ExternalInput").ap()
    wt = nc.dram_tensor("wt", [IC, NTAP, OC], f32, kind="ExternalInput").ap()
    y = nc.dram_tensor("y", [NPC, OC, H, W], f32, kind="ExternalOutput").ap()

    xr = x.rearrange("n (c p) h w -> n c p h w", p=128)
    str_ = stt.rearrange("(sc p) n -> p sc n", p=128)
    fcr = fcwt.rearrange("(sc p) i -> sc p i", p=128)
    fbr = fcb.rearrange("(c p) -> p c", p=128)
    wr = wt.rearrange("(c p) t o -> c p t o", p=128)
    yr = y.rearrange("n (o p) h w -> n o p (h w)", p=128)

    with tile.TileContext(nc) as tc:
        import contextlib
        with contextlib.ExitStack() as ctx:
            singles = ctx.enter_context(tc.tile_pool(name="singles", bufs=1))
            small = ctx.enter_context(tc.tile_pool(name="small", bufs=4))
            outp = ctx.enter_context(tc.tile_pool(name="outp", bufs=4))
            psc = ctx.enter_context(tc.tile_pool(name="psc", bufs=6, space="PSUM"))
            pss = ctx.enter_context(tc.tile_pool(name="pss", bufs=2, space="PSUM"))
            if staged:
                stage = ctx.enter_context(tc.tile_pool(name="stage", bufs=8))
                xnp = ctx.enter_context(tc.tile_pool(name="xnp", bufs=4))

            # ---- persistent SBUF tensors ----
            st_sb = singles.tile([128, SC, NPC], f32)
            fb_sb = singles.tile([128, PC], f32)
            modT_sb = singles.tile([128, PC, NPC], f32)
            mod2T_sb = singles.tile([128, PC, NPC], f32)
            A_sb = singles.tile([128, PC, OC], f32)
            demodT_sb = singles.tile([128, OCC, NPC], f32)
            wt_sb = singles.tile([128, PC, NTAP, OC], conv_dt)
            xs_pad = singles.tile([128, NPC, PC, HP, WP], conv_dt)
            eps_sb = singles.tile([128, 1], f32)
            fcw_sb = singles.tile([128, SC, IC], f32)
            w2_sb = singles.tile([128, NTAP, OC], f32)

            nc.vector.memset(eps_sb[:], float(DEMOD_EPS))

            # ---- PE warm-up: ~3.5us of dummy matmuls on zeros so the HAM
            #      clock gate reaches 8/8 before the real work arrives ----
            warm_sb = singles.tile([128, 512], conv_dt)
            if mode == "f32r":
                nc.gpsimd.memset(warm_sb[:].bitcast(f32), 0.0)
            else:
                nc.gpsimd.memset(warm_sb[:], 0.0)
            wps = pss.tile([128, 512], f32, tag="pmm")
            NWARM = 20
            for i in range(NWARM):
                nc.tensor.matmul(wps[:], warm_sb[:, 0:128], warm_sb[:],
                                 start=(i == 0), stop=(i == NWARM - 1))

            # ---- style modulation: modT[ic, n] ----
            nc.sync.dma_start(st_sb[:], str_)
            nc.sync.dma_start(fb_sb[:], fbr)
            for sc in range(SC):
                nc.sync.dma_start(fcw_sb[:, sc], fcr[sc])
            nc.vector.tensor_scalar_add(fb_sb[:], fb_sb[:], 1.0)
            for c in range(PC):
                pm = pss.tile([128, NPC], f32, tag="pmm")
                for sc in range(SC):
                    nc.tensor.matmul(
                        pm[:], fcw_sb[:, sc, c * 128:(c + 1) * 128],
                        st_sb[:, sc], start=(sc == 0), stop=(sc == SC - 1))
                nc.scalar.activation(
                    modT_sb[:, c], pm[:], mybir.ActivationFunctionType.Identity,
                    bias=fb_sb[:, c:c + 1], scale=FC_SCALER)
            nc.vector.tensor_mul(mod2T_sb[:], modT_sb[:], modT_sb[:])

            # ---- padded, modulated input images (per-chunk memset so the
            #      first chunk is ready fast) ----
            def prep_x(n, c):
                pv = xs_pad[:, n, c]
                if mode == "f32r":
                    # memset can't encode f32r; zero bits are identical
                    nc.gpsimd.memset(pv.bitcast(f32), 0.0)
                else:
                    nc.gpsimd.memset(pv, 0.0)
                iv = pv[:, 1:H + 1, 1:W + 1]
                if staged:
                    xn = xnp.tile([128, H, W], f32, tag="xn")
                    nc.sync.dma_start(xn[:], xr[n, c])
                    nc.scalar.mul(iv, xn[:], modT_sb[:, c, n:n + 1])
                else:
                    nc.sync.dma_start(iv, xr[n, c])
                    nc.scalar.mul(iv, iv, modT_sb[:, c, n:n + 1])

            # ---- weights (+ cast) and A[ic, oc] = sum_taps w^2 ----
            # per-tap staging so the PE can start consuming weights while
            # later taps are still in flight; x chunks interleave with the
            # weight chunks in PE consumption order
            for c in range(PC):
                prep_x(0, c)
                for t in range(NTAP):
                    if staged:
                        ws = stage.tile([128, OC], f32, tag="ws")
                        nc.sync.dma_start(ws[:], wr[c][:, t])
                        # alternate the rounding cast between ACT and DVE so
                        # the first chunk's taps emerge faster than the PE
                        # consumes them
                        if t % 2 == 0:
                            nc.scalar.copy(wt_sb[:, c, t], ws[:])
                        else:
                            nc.vector.tensor_copy(wt_sb[:, c, t], ws[:])
                        nc.vector.tensor_mul(w2_sb[:, t], ws[:], ws[:])
                    else:
                        nc.sync.dma_start(wt_sb[:, c, t], wr[c][:, t])
                        nc.vector.tensor_mul(w2_sb[:, t], wt_sb[:, c, t],
                                             wt_sb[:, c, t])
                # contiguous tree reduction over the 9 tap planes
                nc.vector.tensor_add(w2_sb[:, 0], w2_sb[:, 0], w2_sb[:, 8])
                nc.vector.tensor_add(w2_sb[:, 0:4], w2_sb[:, 0:4], w2_sb[:, 4:8])
                nc.vector.tensor_add(w2_sb[:, 0:2], w2_sb[:, 0:2], w2_sb[:, 2:4])
                nc.vector.tensor_add(A_sb[:, c], w2_sb[:, 0], w2_sb[:, 1])
            for c in range(PC):
                prep_x(1, c)

            # ---- demodulation scale: 1/sqrt(sumsq + eps') ----
            for o in range(OCC):
                ps2 = pss.tile([128, NPC], f32, tag="pmm")
                for c in range(PC):
                    nc.tensor.matmul(
                        ps2[:], A_sb[:, c, o * 128:(o + 1) * 128],
                        mod2T_sb[:, c], start=(c == 0), stop=(c == PC - 1))
                sqt = small.tile([128, NPC], f32, tag="sqt")
                nc.scalar.activation(sqt[:], ps2[:],
                                     mybir.ActivationFunctionType.Sqrt,
                                     bias=eps_sb[:])
                nc.vector.reciprocal(demodT_sb[:, o], sqt[:])

            # ---- the conv: 2 samples x 4 oc-chunks, two half-image PSUM
            #      groups accumulating together so consecutive matmuls share
            #      the stationary weights (gives LDWEIGHTS a 2-MM window) ----
            NH = H // HALF
            for n in range(NPC):
                for o in range(OCC):
                    ps = [psc.tile([128, HALF * W], f32, tag="ps", name=f"ps{h}")
                          for h in range(NH)]
                    i = 0
                    for c in range(PC):
                        for ky in range(K):
                            for kx in range(K):
                                for h in range(NH):
                                    nc.tensor.matmul(
                                        ps[h][:],
                                        wt_sb[:, c, ky * K + kx,
                                              o * 128:(o + 1) * 128],
                                        xs_pad[:, n, c,
                                               ky + h * HALF:ky + h * HALF + HALF,
                                               kx:kx + W],
                                        start=(i == 0), stop=(i >= 2 * PC * NTAP - 2))
                                i += 2
                    early = (n == 0 and o < 2)
                    for h in range(NH):
                        ob = outp.tile([128, HALF * W], f32, tag="ob")
                        if early:
                            # demod isn't ready yet; drain the PSUM bank
                            # immediately and scale later
                            nc.scalar.copy(ob[:], ps[h][:])
                            nc.vector.tensor_scalar_mul(ob[:], ob[:],
                                                        demodT_sb[:, o, n:n + 1])
                        else:
                            nc.vector.tensor_scalar_mul(ob[:], ps[h][:],
                                                        demodT_sb[:, o, n:n + 1])
                        nc.sync.dma_start(
                            yr[n, o][:, h * HALF * W:(h + 1) * HALF * W], ob[:])

    nc.finalize()
    _NC_CACHE[mode] = nc
    return nc


def _shard_inputs(x, style, weight, fc_weight, fc_bias):
    f = np.float32
    wt_host = np.ascontiguousarray(
        weight.astype(f).transpose(1, 2, 3, 0).reshape(IC, NTAP, OC))
    fcwt_host = np.ascontiguousarray(fc_weight.astype(f).T)
    fcb_host = np.ascontiguousarray(fc_bias.astype(f))
    in_maps = []
    for i in range(N_CORES):
        sl = slice(i * NPC, (i + 1) * NPC)
        in_maps.append({
            "x": np.ascontiguousarray(x[sl].astype(f)),
            "stt": np.ascontiguousarray(style[sl].astype(f).T),
            "fcwt": fcwt_host,
            "fcb": fcb_host,
            "wt": wt_host,
        })
    return in_maps


def kernel(x, style, weight, fc_weight, fc_bias):
    global LAST_RESULT
    x = np.asarray(x)
    style = np.asarray(style)
    weight = np.asarray(weight)
    fc_weight = np.asarray(fc_weight)
    fc_bias = np.asarray(fc_bias)

    nc = build_nc()
    in_maps = _shard_inputs(x, style, weight, fc_weight, fc_bias)
    res = bass_utils.run_bass_kernel_spmd(
        nc, in_maps, core_ids=list(range(N_CORES)))
    LAST_RESULT = res
    out = np.concatenate([res.results[i]["y"] for i in range(N_CORES)], axis=0)
    return out.astype(np.float32)


# revision 24
# speedup vs baseline: 3.1922x; 1.0161x over previous
"""EqualizedModulatedConv2d (StyleGAN2-style modulated conv) on 8 Trainium2 cores.

Reference computation (per sample n):
    mod[n, ic]  = (style[n] @ fc_weight.T) * FC_SCALER + fc_bias + 1
    w[n]        = WEIGHT_SCALER * weight * mod[n, :, None, None]          # [oC, iC, 3, 3]
    demod[n,oc] = rsqrt(sum_{ic,kh,kw} w^2 + 1e-8)
    out[n]      = conv2d(x[n], w[n] * demod[n, :, None, None, None], pad=1)

Key identity used on device: the conv is linear, so
    out[n, oc] = (WEIGHT_SCALER * demod[n, oc]) * conv2d(x[n] * mod[n, ic], weight)
with
    WEIGHT_SCALER * demod[n, oc] = 1 / sqrt(sumsq[n, oc] + 1e-8 / WEIGHT_SCALER^2)
    sumsq[n, oc] = sum_ic A[ic, oc] * mod[n, ic]^2,   A[ic, oc] = sum_taps weight^2

Sharding: data-parallel over N (16 samples / 8 cores = 2 samples per core);
weight / fc_weight replicated. The conv itself runs as 3x3 = 9 shifted matmuls
on the PE array, contracting over iC in 4 chunks of 128 partitions, from a
zero-padded 34x34 SBUF image buffer.
"""

import os

import numpy as np

import concourse.bass as bass
import concourse.tile as tile
from concourse import bacc, mybir
import concourse.bass_utils as bass_utils

# ---- problem constants (hardcoded per the harness contract) ----
N, IC, OC, K, SDIM, H, W = 16, 512, 512, 3, 512, 32, 32
N_CORES = 8
NPC = N // N_CORES            # samples per core = 2
PC = IC // 128                # ic chunks = 4
OCC = OC // 128               # oc chunks = 4
SC = SDIM // 128              # sdim chunks = 4
NTAP = K * K                  # 9
HP, WP = H + 2, W + 2         # 34, 34 padded
FC_SCALER = 1.0 / np.sqrt(SDIM)
WEIGHT_SCALER = 1.0 / np.sqrt(IC * K * K)
DEMOD_EPS = 1e-8 / (WEIGHT_SCALER * WEIGHT_SCALER)   # 1e-8 * IC * K * K
HALF = 16                     # rows per conv matmul (16*32 = 512 = PSUM bank)

# matmul operand mode: "f32" (exact, 4 cyc/row), "f32r" (fast fp32, 1 cyc/row),
# "bf16" (fast + halved weight-load cost, lower precision)
MODE = os.environ.get("TRN_CONV_DTYPE", "f32r")

_NC_CACHE = {}
LAST_RESULT = None  # test.py reads exec_time_ns off this


def build_nc(mode=None):
    mode = mode or MODE
    if mode in _NC_CACHE:
        return _NC_CACHE[mode]

    f32 = mybir.dt.float32
    conv_dt = {"bf16": mybir.dt.bfloat16,
               "f32r": mybir.dt.float32r,
               "f32": f32}[mode]
    staged = mode != "f32"

    nc = bacc.Bacc("TRN2", target_bir_lowering=False, debug=False,
                   num_devices=N_CORES)

    x = nc.dram_tensor("x", [NPC, IC, H, W], f32, kind="ExternalInput").ap()
    stt = nc.dram_tensor("stt", [SDIM, NPC], f32, kind="ExternalInput").ap()
    fcwt = nc.dram_tensor("fcwt", [SDIM, IC], f32, kind="ExternalInput").ap()
    fcb = nc.dram_tensor("fcb", [IC], f32, kind="ExternalInput").ap()
    wt = nc.dram_tensor("wt", [IC, NTAP, OC], f32, kind="ExternalInput").ap()
    y = nc.dram_tensor("y", [NPC, OC, H, W], f32, kind="ExternalOutput").ap()

    xr = x.rearrange("n (c p) h w -> n c p h w", p=128)
    str_ = stt.rearrange("(sc p) n -> p sc n", p=128)
    fcr = fcwt.rearrange("(sc p) i -> sc p i", p=128)
    fbr = fcb.rearrange("(c p) -> p c", p=128)
    wr = wt.rearrange("(c p) t o -> c p t o", p=128)
    yr = y.rearrange("n (o p) h w -> n o p (h w)", p=128)

    with tile.TileContext(nc) as tc:
        import contextlib
        with contextlib.ExitStack() as ctx:
            singles = ctx.enter_context(tc.tile_pool(name="singles", bufs=1))
            small = ctx.enter_context(tc.tile_pool(name="small", bufs=4))
            outp = ctx.enter_context(tc.tile_pool(name="outp", bufs=4))
            psc = ctx.enter_context(tc.tile_pool(name="psc", bufs=6, space="PSUM"))
            pss = ctx.enter_context(tc.tile_pool(name="pss", bufs=2, space="PSUM"))
            if staged:
                stage = ctx.enter_context(tc.tile_pool(name="stage", bufs=8))
                xnp = ctx.enter_context(tc.tile_pool(name="xnp", bufs=4))

            # ---- persistent SBUF tensors ----
            st_sb = singles.tile([128, SC, NPC], f32)
            fb_sb = singles.tile([128, PC], f32)
            modT_sb = singles.tile([128, PC, NPC], f32)
            mod2T_sb = singles.tile([128, PC, NPC], f32)
            A_sb = singles.tile([128, PC, OC], f32)
            demodT_sb = singles.tile([128, OCC, NPC], f32)
            wt_sb = singles.tile([128, PC, NTAP, OC], conv_dt)
            xs_pad = singles.tile([128, NPC, PC, HP, WP], conv_dt)
            eps_sb = singles.tile([128, 1], f32)
            fcw_sb = singles.tile([128, SC, IC], f32)
            w2_sb = singles.tile([128, NTAP, OC], f32)

            nc.vector.memset(eps_sb[:], float(DEMOD_EPS))

            # ---- PE warm-up: ~3.5us of dummy matmuls on zeros so the HAM
            #      clock gate reaches 8/8 before the real work arrives ----
            warm_sb = singles.tile([128, 512], conv_dt)
            if mode == "f32r":
                nc.gpsimd.memset(warm_sb[:].bitcast(f32), 0.0)
            else:
                nc.gpsimd.memset(warm_sb[:], 0.0)
            wps = pss.tile([128, 512], f32, tag="pmm")
            NWARM = 20
            for i in range(NWARM):
                nc.tensor.matmul(wps[:], warm_sb[:, 0:128], warm_sb[:],
                                 start=(i == 0), stop=(i == NWARM - 1))

            # ---- style modulation: modT[ic, n] ----
            nc.sync.dma_start(st_sb[:], str_)
            nc.sync.dma_start(fb_sb[:], fbr)
            for sc in range(SC):
                nc.sync.dma_start(fcw_sb[:, sc], fcr[sc])
            nc.vector.tensor_scalar_add(fb_sb[:], fb_sb[:], 1.0)
            for c in range(PC):
                pm = pss.tile([128, NPC], f32, tag="pmm")
                for sc in range(SC):
                    nc.tensor.matmul(
                        pm[:], fcw_sb[:, sc, c * 128:(c + 1) * 128],
                        st_sb[:, sc], start=(sc == 0), stop=(sc == SC - 1))
                nc.scalar.activation(
                    modT_sb[:, c], pm[:], mybir.ActivationFunctionType.Identity,
                    bias=fb_sb[:, c:c + 1], scale=FC_SCALER)
            nc.vector.tensor_mul(mod2T_sb[:], modT_sb[:], modT_sb[:])

            # ---- padded, modulated input images (per-chunk memset so the
            #      first chunk is ready fast) ----
            def prep_x(n, c):
                pv = xs_pad[:, n, c]
                if mode == "f32r":
                    # memset can't encode f32r; zero bits are identical
                    nc.gpsimd.memset(pv.bitcast(f32), 0.0)
                else:
                    nc.gpsimd.memset(pv, 0.0)
                iv = pv[:, 1:H + 1, 1:W + 1]
                if staged:
                    xn = xnp.tile([128, H, W], f32, tag="xn")
                    nc.sync.dma_start(xn[:], xr[n, c])
                    nc.scalar.mul(iv, xn[:], modT_sb[:, c, n:n + 1])
                else:
                    nc.sync.dma_start(iv, xr[n, c])
                    nc.scalar.mul(iv, iv, modT_sb[:, c, n:n + 1])

            # ---- weights (+ cast) and A[ic, oc] = sum_taps w^2 ----
            # per-tap staging so the PE can start consuming weights while
            # later taps are still in flight; x chunks interleave with the
            # weight chunks in PE consumption order
            for c in range(PC):
                prep_x(0, c)
                for t in range(NTAP):
                    if staged:
                        ws = stage.tile([128, OC], f32, tag="ws")
                        nc.sync.dma_start(ws[:], wr[c][:, t])
                        # alternate the rounding cast between ACT and DVE so
                        # the first chunk's taps emerge faster than the PE
                        # consumes them
                        if t % 2 == 0:
                            nc.scalar.copy(wt_sb[:, c, t], ws[:])
                        else:
                            nc.vector.tensor_copy(wt_sb[:, c, t], ws[:])
                        nc.vector.tensor_mul(w2_sb[:, t], ws[:], ws[:])
                    else:
                        nc.sync.dma_start(wt_sb[:, c, t], wr[c][:, t])
                        nc.vector.tensor_mul(w2_sb[:, t], wt_sb[:, c, t],
                                             wt_sb[:, c, t])
                # contiguous tree reduction over the 9 tap planes
                nc.vector.tensor_add(w2_sb[:, 0], w2_sb[:, 0], w2_sb[:, 8])
                nc.vector.tensor_add(w2_sb[:, 0:4], w2_sb[:, 0:4], w2_sb[:, 4:8])
                nc.vector.tensor_add(w2_sb[:, 0:2], w2_sb[:, 0:2], w2_sb[:, 2:4])
                nc.vector.tensor_add(A_sb[:, c], w2_sb[:, 0], w2_sb[:, 1])
            for c in range(PC):
                prep_x(1, c)

            # ---- demodulation scale: 1/sqrt(sumsq + eps') ----
            for o in range(OCC):
                ps2 = pss.tile([128, NPC], f32, tag="pmm")
                for c in range(PC):
                    nc.tensor.matmul(
                        ps2[:], A_sb[:, c, o * 128:(o + 1) * 128],
                        mod2T_sb[:, c], start=(c == 0), stop=(c == PC - 1))
                sqt = small.tile([128, NPC], f32, tag="sqt")
                nc.scalar.activation(sqt[:], ps2[:],
                                     mybir.ActivationFunctionType.Sqrt,
                                     bias=eps_sb[:])
                nc.vector.reciprocal(demodT_sb[:, o], sqt[:])

            # ---- the conv: 2 samples x 4 oc-chunks, two half-image PSUM
            #      groups accumulating together so consecutive matmuls share
            #      the stationary weights (gives LDWEIGHTS a 2-MM window) ----
            NH = H // HALF
            for n in range(NPC):
                for op in [(0, 1), (2, 3)]:
                    # 4 PSUM banks (2 oc-chunks x 2 half-images) accumulate
                    # together: 4x the PE work per weight-chunk arrival, so
                    # the early chunks don't outrun HBM
                    ps = {(o, h): psc.tile([128, HALF * W], f32, tag="ps",
                                           name=f"ps{o}_{h}")
                          for o in op for h in range(NH)}
                    first = True
                    for c in range(PC):
                        for ky in range(K):
                            for kx in range(K):
                                last = (c == PC - 1 and ky == K - 1
                                        and kx == K - 1)
                                for o in op:
                                    for h in range(NH):
                                        nc.tensor.matmul(
                                            ps[o, h][:],
                                            wt_sb[:, c, ky * K + kx,
                                                  o * 128:(o + 1) * 128],
                                            xs_pad[:, n, c,
                                                   ky + h * HALF:ky + h * HALF + HALF,
                                                   kx:kx + W],
                                            start=first, stop=last)
                                first = False
                    early = (n == 0 and op == (0, 1))
                    for o in op:
                        for h in range(NH):
                            ob = outp.tile([128, HALF * W], f32, tag="ob")
                            if early:
                                # demod isn't ready yet; drain the PSUM bank
                                # immediately and scale later
                                nc.scalar.copy(ob[:], ps[o, h][:])
                                nc.vector.tensor_scalar_mul(
                                    ob[:], ob[:], demodT_sb[:, o, n:n + 1])
                            else:
                                nc.vector.tensor_scalar_mul(
                                    ob[:], ps[o, h][:], demodT_sb[:, o, n:n + 1])
                            nc.sync.dma_start(
                                yr[n, o][:, h * HALF * W:(h + 1) * HALF * W],
                                ob[:])

    nc.finalize()
    _NC_CACHE[mode] = nc
    return nc


def _shard_inputs(x, style, weight, fc_weight, fc_bias):
    f = np.float32
    wt_host = np.ascontiguousarray(
        weight.astype(f).transpose(1, 2, 3, 0).reshape(IC, NTAP, OC))
    fcwt_host = np.ascontiguousarray(fc_weight.astype(f).T)
    fcb_host = np.ascontiguousarray(fc_bias.astype(f))
    in_maps = []
    for i in range(N_CORES):
        sl = slice(i * NPC, (i + 1) * NPC)
        in_maps.append({
            "x": np.ascontiguousarray(x[sl].astype(f)),
            "stt": np.ascontiguousarray(style[sl].astype(f).T),
            "fcwt": fcwt_host,
            "fcb": fcb_host,
            "wt": wt_host,
        })
    return in_maps


def kernel(x, style, weight, fc_weight, fc_bias):
    global LAST_RESULT
    x = np.asarray(x)
    style = np.asarray(style)
    weight = np.asarray(weight)
    fc_weight = np.asarray(fc_weight)
    fc_bias = np.asarray(fc_bias)

    nc = build_nc()
    in_maps = _shard_inputs(x, style, weight, fc_weight, fc_bias)
    res = bass_utils.run_bass_kernel_spmd(
        nc, in_maps, core_ids=list(range(N_CORES)))
    LAST_RESULT = res
    out = np.concatenate([res.results[i]["y"] for i in range(N_CORES)], axis=0)
    return out.astype(np.float32)


# revision 26
# speedup vs baseline: 3.2195x; 1.0086x over previous
"""EqualizedModulatedConv2d (StyleGAN2-style modulated conv) on 8 Trainium2 cores.

Reference computation (per sample n):
    mod[n, ic]  = (style[n] @ fc_weight.T) * FC_SCALER + fc_bias + 1
    w[n]        = WEIGHT_SCALER * weight * mod[n, :, None, None]          # [oC, iC, 3, 3]
    demod[n,oc] = rsqrt(sum_{ic,kh,kw} w^2 + 1e-8)
    out[n]      = conv2d(x[n], w[n] * demod[n, :, None, None, None], pad=1)

Key identity used on device: the conv is linear, so
    out[n, oc] = (WEIGHT_SCALER * demod[n, oc]) * conv2d(x[n] * mod[n, ic], weight)
with
    WEIGHT_SCALER * demod[n, oc] = 1 / sqrt(sumsq[n, oc] + 1e-8 / WEIGHT_SCALER^2)
    sumsq[n, oc] = sum_ic A[ic, oc] * mod[n, ic]^2,   A[ic, oc] = sum_taps weight^2

Sharding: data-parallel over N (16 samples / 8 cores = 2 samples per core);
weight / fc_weight replicated. The conv itself runs as 3x3 = 9 shifted matmuls
on the PE array, contracting over iC in 4 chunks of 128 partitions, from a
zero-padded 34x34 SBUF image buffer.
"""

import os

import numpy as np

import concourse.bass as bass
import concourse.tile as tile
from concourse import bacc, mybir
import concourse.bass_utils as bass_utils

# ---- problem constants (hardcoded per the harness contract) ----
N, IC, OC, K, SDIM, H, W = 16, 512, 512, 3, 512, 32, 32
N_CORES = 8
NPC = N // N_CORES            # samples per core = 2
PC = IC // 128                # ic chunks = 4
OCC = OC // 128               # oc chunks = 4
SC = SDIM // 128              # sdim chunks = 4
NTAP = K * K                  # 9
HP, WP = H + 2, W + 2         # 34, 34 padded
FC_SCALER = 1.0 / np.sqrt(SDIM)
WEIGHT_SCALER = 1.0 / np.sqrt(IC * K * K)
DEMOD_EPS = 1e-8 / (WEIGHT_SCALER * WEIGHT_SCALER)   # 1e-8 * IC * K * K
HALF = 16                     # rows per conv matmul (16*32 = 512 = PSUM bank)

# matmul operand mode: "f32" (exact, 4 cyc/row), "f32r" (fast fp32, 1 cyc/row),
# "bf16" (fast + halved weight-load cost, lower precision)
MODE = os.environ.get("TRN_CONV_DTYPE", "f32r")

_NC_CACHE = {}
LAST_RESULT = None  # test.py reads exec_time_ns off this


def build_nc(mode=None):
    mode = mode or MODE
    if mode in _NC_CACHE:
        return _NC_CACHE[mode]

    f32 = mybir.dt.float32
    conv_dt = {"bf16": mybir.dt.bfloat16,
               "f32r": mybir.dt.float32r,
               "f32": f32}[mode]
    staged = mode != "f32"

    nc = bacc.Bacc("TRN2", target_bir_lowering=False, debug=False,
                   num_devices=N_CORES)

    x = nc.dram_tensor("x", [NPC, IC, H, W], f32, kind="ExternalInput").ap()
    stt = nc.dram_tensor("stt", [SDIM, NPC], f32, kind="ExternalInput").ap()
    fcwt = nc.dram_tensor("fcwt", [SDIM, IC], f32, kind="ExternalInput").ap()
    fcb = nc.dram_tensor("fcb", [IC], f32, kind="ExternalInput").ap()
    wt = nc.dram_tensor("wt", [IC, NTAP, OC], f32, kind="ExternalInput").ap()
    y = nc.dram_tensor("y", [NPC, OC, H, W], f32, kind="ExternalOutput").ap()

    xr = x.rearrange("n (c p) h w -> n c p h w", p=128)
    str_ = stt.rearrange("(sc p) n -> p sc n", p=128)
    fcr = fcwt.rearrange("(sc p) i -> sc p i", p=128)
    fbr = fcb.rearrange("(c p) -> p c", p=128)
    wr = wt.rearrange("(c p) t o -> c p t o", p=128)
    yr = y.rearrange("n (o p) h w -> n o p (h w)", p=128)

    with tile.TileContext(nc) as tc:
        import contextlib
        with contextlib.ExitStack() as ctx:
            singles = ctx.enter_context(tc.tile_pool(name="singles", bufs=1))
            small = ctx.enter_context(tc.tile_pool(name="small", bufs=4))
            outp = ctx.enter_context(tc.tile_pool(name="outp", bufs=4))
            psc = ctx.enter_context(tc.tile_pool(name="psc", bufs=6, space="PSUM"))
            pss = ctx.enter_context(tc.tile_pool(name="pss", bufs=2, space="PSUM"))
            if staged:
                stage = ctx.enter_context(tc.tile_pool(name="stage", bufs=8))
                xnp = ctx.enter_context(tc.tile_pool(name="xnp", bufs=4))

            # ---- persistent SBUF tensors ----
            st_sb = singles.tile([128, SC, NPC], f32)
            fb_sb = singles.tile([128, PC], f32)
            modT_sb = singles.tile([128, PC, NPC], f32)
            # sumsq matmul operands in f32r so their 16 mid-stream LDWEIGHTS
            # don't pay the slow fp32 weight-load path (precision impact on
            # demod ~1e-4, at the conv's own noise floor)
            tiny_dt = mybir.dt.float32r if mode == "f32r" else f32
            mod2T_sb = singles.tile([128, PC, NPC], tiny_dt)
            A_sb = singles.tile([128, PC, OC], tiny_dt)
            demodT_sb = singles.tile([128, OCC, NPC], f32)
            wt_sb = singles.tile([128, PC, NTAP, OC], conv_dt)
            xs_pad = singles.tile([128, NPC, PC, HP, WP], conv_dt)
            eps_sb = singles.tile([128, 1], f32)
            fcw_sb = singles.tile([128, SC, IC], f32)
            w2_sb = singles.tile([128, NTAP, OC], f32)

            nc.vector.memset(eps_sb[:], float(DEMOD_EPS))

            # ---- PE warm-up: ~3.5us of dummy matmuls on zeros so the HAM
            #      clock gate reaches 8/8 before the real work arrives ----
            warm_sb = singles.tile([128, 512], conv_dt)
            if mode == "f32r":
                nc.gpsimd.memset(warm_sb[:].bitcast(f32), 0.0)
            else:
                nc.gpsimd.memset(warm_sb[:], 0.0)
            wps = pss.tile([128, 512], f32, tag="pmm")
            NWARM = 20
            for i in range(NWARM):
                nc.tensor.matmul(wps[:], warm_sb[:, 0:128], warm_sb[:],
                                 start=(i == 0), stop=(i == NWARM - 1))

            # ---- style modulation: modT[ic, n] ----
            nc.sync.dma_start(st_sb[:], str_)
            nc.sync.dma_start(fb_sb[:], fbr)
            for sc in range(SC):
                nc.sync.dma_start(fcw_sb[:, sc], fcr[sc])
            nc.vector.tensor_scalar_add(fb_sb[:], fb_sb[:], 1.0)
            for c in range(PC):
                pm = pss.tile([128, NPC], f32, tag="pmm")
                for sc in range(SC):
                    nc.tensor.matmul(
                        pm[:], fcw_sb[:, sc, c * 128:(c + 1) * 128],
                        st_sb[:, sc], start=(sc == 0), stop=(sc == SC - 1))
                nc.scalar.activation(
                    modT_sb[:, c], pm[:], mybir.ActivationFunctionType.Identity,
                    bias=fb_sb[:, c:c + 1], scale=FC_SCALER)
            nc.vector.tensor_mul(mod2T_sb[:], modT_sb[:], modT_sb[:])

            # ---- padded, modulated input images (per-chunk memset so the
            #      first chunk is ready fast) ----
            def prep_x(n, c):
                pv = xs_pad[:, n, c]
                if mode == "f32r":
                    # memset can't encode f32r; zero bits are identical
                    nc.gpsimd.memset(pv.bitcast(f32), 0.0)
                else:
                    nc.gpsimd.memset(pv, 0.0)
                iv = pv[:, 1:H + 1, 1:W + 1]
                if staged:
                    xn = xnp.tile([128, H, W], f32, tag="xn")
                    nc.sync.dma_start(xn[:], xr[n, c])
                    nc.scalar.mul(iv, xn[:], modT_sb[:, c, n:n + 1])
                else:
                    nc.sync.dma_start(iv, xr[n, c])
                    nc.scalar.mul(iv, iv, modT_sb[:, c, n:n + 1])

            # ---- weights (+ cast) and A[ic, oc] = sum_taps w^2 ----
            # per-tap staging so the PE can start consuming weights while
            # later taps are still in flight; x chunks interleave with the
            # weight chunks in PE consumption order
            for c in range(PC):
                prep_x(0, c)
                for t in range(NTAP):
                    if staged:
                        ws = stage.tile([128, OC], f32, tag="ws")
                        nc.sync.dma_start(ws[:], wr[c][:, t])
                        # alternate the rounding cast between ACT and DVE so
                        # the first chunk's taps emerge faster than the PE
                        # consumes them
                        if t % 2 == 0:
                            nc.scalar.copy(wt_sb[:, c, t], ws[:])
                        else:
                            nc.vector.tensor_copy(wt_sb[:, c, t], ws[:])
                        nc.vector.tensor_mul(w2_sb[:, t], ws[:], ws[:])
                    else:
                        nc.sync.dma_start(wt_sb[:, c, t], wr[c][:, t])
                        nc.vector.tensor_mul(w2_sb[:, t], wt_sb[:, c, t],
                                             wt_sb[:, c, t])
                # contiguous tree reduction over the 9 tap planes
                nc.vector.tensor_add(w2_sb[:, 0], w2_sb[:, 0], w2_sb[:, 8])
                nc.vector.tensor_add(w2_sb[:, 0:4], w2_sb[:, 0:4], w2_sb[:, 4:8])
                nc.vector.tensor_add(w2_sb[:, 0:2], w2_sb[:, 0:2], w2_sb[:, 2:4])
                nc.vector.tensor_add(A_sb[:, c], w2_sb[:, 0], w2_sb[:, 1])
            for c in range(PC):
                prep_x(1, c)

            # ---- demodulation scale: 1/sqrt(sumsq + eps') ----
            for o in range(OCC):
                ps2 = pss.tile([128, NPC], f32, tag="pmm")
                for c in range(PC):
                    nc.tensor.matmul(
                        ps2[:], A_sb[:, c, o * 128:(o + 1) * 128],
                        mod2T_sb[:, c], start=(c == 0), stop=(c == PC - 1))
                sqt = small.tile([128, NPC], f32, tag="sqt")
                nc.scalar.activation(sqt[:], ps2[:],
                                     mybir.ActivationFunctionType.Sqrt,
                                     bias=eps_sb[:])
                nc.vector.reciprocal(demodT_sb[:, o], sqt[:])

            # ---- the conv: 2 samples x 4 oc-chunks, two half-image PSUM
            #      groups accumulating together so consecutive matmuls share
            #      the stationary weights (gives LDWEIGHTS a 2-MM window) ----
            NH = H // HALF
            for n in range(NPC):
                for op in [(0, 1), (2, 3)]:
                    # 4 PSUM banks (2 oc-chunks x 2 half-images) accumulate
                    # together: 4x the PE work per weight-chunk arrival, so
                    # the early chunks don't outrun HBM
                    ps = {(o, h): psc.tile([128, HALF * W], f32, tag="ps",
                                           name=f"ps{o}_{h}")
                          for o in op for h in range(NH)}
                    first = True
                    for c in range(PC):
                        for ky in range(K):
                            for kx in range(K):
                                last = (c == PC - 1 and ky == K - 1
                                        and kx == K - 1)
                                for o in op:
                                    for h in range(NH):
                                        nc.tensor.matmul(
                                            ps[o, h][:],
                                            wt_sb[:, c, ky * K + kx,
                                                  o * 128:(o + 1) * 128],
                                            xs_pad[:, n, c,
                                                   ky + h * HALF:ky + h * HALF + HALF,
                                                   kx:kx + W],
                                            start=first, stop=last)
                                first = False
                    early = (n == 0 and op == (0, 1))
                    for o in op:
                        for h in range(NH):
                            ob = outp.tile([128, HALF * W], f32, tag="ob")
                            if early:
                                # demod isn't ready yet; drain the PSUM bank
                                # immediately and scale later
                                nc.scalar.copy(ob[:], ps[o, h][:])
                                nc.vector.tensor_scalar_mul(
                                    ob[:], ob[:], demodT_sb[:, o, n:n + 1])
                            else:
                                nc.vector.tensor_scalar_mul(
                                    ob[:], ps[o, h][:], demodT_sb[:, o, n:n + 1])
                            nc.sync.dma_start(
                                yr[n, o][:, h * HALF * W:(h + 1) * HALF * W],
                                ob[:])

    nc.finalize()
    _NC_CACHE[mode] = nc
    return nc


def _shard_inputs(x, style, weight, fc_weight, fc_bias):
    f = np.float32
    wt_host = np.ascontiguousarray(
        weight.astype(f).transpose(1, 2, 3, 0).reshape(IC, NTAP, OC))
    fcwt_host = np.ascontiguousarray(fc_weight.astype(f).T)
    fcb_host = np.ascontiguousarray(fc_bias.astype(f))
    in_maps = []
    for i in range(N_CORES):
        sl = slice(i * NPC, (i + 1) * NPC)
        in_maps.append({
            "x": np.ascontiguousarray(x[sl].astype(f)),
            "stt": np.ascontiguousarray(style[sl].astype(f).T),
            "fcwt": fcwt_host,
            "fcb": fcb_host,
            "wt": wt_host,
        })
    return in_maps


def kernel(x, style, weight, fc_weight, fc_bias):
    global LAST_RESULT
    x = np.asarray(x)
    style = np.asarray(style)
    weight = np.asarray(weight)
    fc_weight = np.asarray(fc_weight)
    fc_bias = np.asarray(fc_bias)

    nc = build_nc()
    in_maps = _shard_inputs(x, style, weight, fc_weight, fc_bias)
    res = bass_utils.run_bass_kernel_spmd(
        nc, in_maps, core_ids=list(range(N_CORES)))
    LAST_RESULT = res
    out = np.concatenate([res.results[i]["y"] for i in range(N_CORES)], axis=0)
    return out.astype(np.float32)
